# revision 27
# baseline (speedup 1.0000x reference)
"""BoundaryLoss kernel for Trainium2 (8 NeuronCores, SPMD data-parallel over batch).

Problem (per reference):
  B=32, W=256, N=8192, d=3
  sq[b,w,n] = |w|^2 - 2 w.p + |p|^2 ; idx = argmin_n sq ; gather closest
  point/normal; dot = (w - cp).cn ; loss = mean_b mean_w exp_relu(dot).

Device strategy per core (4 batches/core):
  D[w,n] = 2 w.p - |p|^2  (argmax_n D == argmin_n sq) via one K=15 float32r
  matmul per [128, 512] tile. f32r keeps 12 mantissa bits and streams at
  1 cycle/column; an exact hi/lo split (x = xh + xl, 12+12 = 24 bits)
  reconstructs full fp32 accuracy from three cross products (wl*pl term
  ~2^-24 is dropped):
    rhs rows 0-2 ph_c (lhsT 2*wh_c), 3-5 pl_c (lhsT 2*wh_c),
    6-8 ph_c (lhsT 2*wl_c), 9-11 sqh_c and 12-14 sql_c (lhsT -1).
  Squares/splits are computed on device in a [96, 256] view and DMA-
  flattened to [3, 8192] rhs rows. Argmin: DVE per-pack reduce_max from
  PSUM + one max_index over the ACT-drained [128, 8192] SBUF block.
  Host does the final (tiny, O(B*W)) gather + dot + exp_relu + mean.
"""

import json

import numpy as np

import concourse.bass as bass
import concourse.mybir as mybir
from concourse.tile import TileContext
from concourse.vector_clock import ScopedClock
from concourse.bass_utils import run_bass_kernel_spmd

# ---------------------------------------------------------------------------
# Workarounds: this container's walrus build rejects >1 sync-wait command per
# instruction. Split excess waits onto NoOps inserted before the instruction.
# ---------------------------------------------------------------------------

_MAX_WAITS = 1
_split_counter = [0]
_installed = [False]


def _split_bir_waits(bir_bytes):
    mod = json.loads(bir_bytes)
    changed = False
    for fn in mod.get("functions", []):
        for blk in fn.get("blocks", []):
            out = []
            for inst in blk.get("instructions", []):
                si = inst.get("sync_info")
                waits = (si or {}).get("on_wait") or []
                if len(waits) > _MAX_WAITS:
                    changed = True
                    extra, keep = waits[:-_MAX_WAITS], waits[-_MAX_WAITS:]
                    for i in range(0, len(extra), _MAX_WAITS):
                        _split_counter[0] += 1
                        out.append({
                            "engine": inst["engine"],
                            "ins": [],
                            "name": f"I-WSPLIT-{_split_counter[0]}",
                            "opcode": "NoOp",
                            "outs": [],
                            "sync_info": {
                                "on_update": [],
                                "on_wait": extra[i:i + _MAX_WAITS],
                            },
                        })
                    si["on_wait"] = keep
                out.append(inst)
            blk["instructions"] = out
    if not changed:
        return bir_bytes
    return json.dumps(mod).encode()


def _patched_drain_and_barrier(self, tick_clock, wait_clock):
    nc = self.nc
    collector = nc.sync.nop(nofuse=True, hint="drain_wait_collector")
    wait_clock.add_sem_waits(
        collector.ins, ScopedClock({None: tick_clock.global_clock})
    )
    si = collector.ins.sync_info
    if si is not None and si.on_wait and len(si.on_wait) > 1:
        waits = list(si.on_wait)
        collector.ins.sync_info = mybir.SyncInfo(
            on_wait=waits[:1], on_update=list(si.on_update)
        )
        for i in range(1, len(waits)):
            extra = nc.sync.nop(nofuse=True, hint=f"drain_wait_{i}")
            extra.ins.sync_info = mybir.SyncInfo(on_wait=[waits[i]], on_update=[])
    nc.sync.drain()
    nc.all_engine_barrier()
    assert self.sems is not None
    popped = nc._tile_sem_poison_stack.pop()
    assert popped is self._sem_poison
    nc.clear_and_free_semaphores(list(self.sems.allocated().values()))
    nc.all_engine_barrier()


def _install_workarounds():
    if _installed[0]:
        return
    _installed[0] = True
    TileContext._drain_and_barrier = _patched_drain_and_barrier

    import concourse.bass2jax as bass2jax

    orig_compile = bass2jax.compile_bir_kernel

    def compile_with_split(ant_bir_str, compile_dir_path, neff_name="file.neff", **kw):
        if isinstance(ant_bir_str, str):
            fixed = _split_bir_waits(ant_bir_str.encode()).decode()
        else:
            fixed = _split_bir_waits(ant_bir_str)
        return orig_compile(fixed, compile_dir_path, neff_name=neff_name, **kw)

    bass2jax.compile_bir_kernel = compile_with_split


# ---------------------------------------------------------------------------
# Problem constants (hardcoded per the harness contract)
# ---------------------------------------------------------------------------

B, W, N, D3 = 32, 256, 8192, 3
N_CORES = 8
B_LOC = B // N_CORES          # 4 batches per core
WCHUNKS = W // 128            # 2 row blocks of 128 waypoints
NCHUNK = 512                  # moving-operand / PSUM bank width
NCHUNKS = N // NCHUNK         # 16

ALPHA = 1.0
BETA = 0.5

_module_cache = {}


def _build_module(loop_reps=None, stage="full"):
    """One NeuronCore's module (SPMD; 4 batches per core).

    f32r (12-bit mantissa) hi/lo-split K=15 matmuls compute the negated
    squared distance D = 2 w.p - |p|^2 at full fp32 accuracy but 1 cycle/
    column (4x faster than fp32 matmul). DVE does a per-pack max from PSUM
    plus one max_index over the assembled SBUF row block; ACT drains
    PSUM->SBUF; prep DMAs are spread over SP/Pool DGE queues.

    loop_reps unrolls the body for amplified timing; stage carves out
    pipeline subsets for bisection benchmarks. Production: defaults."""
    f32 = mybir.dt.float32
    nc = bass.Bass()
    wt = nc.dram_tensor("wt", [B_LOC, 3, W], f32, kind="ExternalInput")
    pt = nc.dram_tensor("pt", [B_LOC, 3, N], f32, kind="ExternalInput")
    idx = nc.dram_tensor("idx", [B_LOC, W], mybir.dt.uint32, kind="ExternalOutput")

    pt_flat = pt.rearrange("b c n -> b (c n)")

    f32r = mybir.dt.float32r
    K15 = 15  # contraction rows per group (f32r hi/lo decomposition)

    with TileContext(nc) as tc:
        with (
            tc.tile_pool(name="rhs", bufs=2) as rhs_pool,
            tc.tile_pool(name="sq", bufs=2) as sq_pool,
            tc.tile_pool(name="lt", bufs=2) as lt_pool,
            tc.tile_pool(name="dmat", bufs=3) as d_pool,
            tc.tile_pool(name="small", bufs=4) as small_pool,
            tc.tile_pool(name="psum", bufs=2, space="PSUM") as psum_pool,
        ):
          for _rep in range(loop_reps or 1):
            for b in range(B_LOC):
                # f32r hi/lo split: x = xh + xl exactly (12+12 mantissa bits).
                # Per row group (partition offset 32i) the K=15 rhs rows are:
                #   0-2: ph_c   (lhsT 2*wh_c)
                #   3-5: pl_c   (lhsT 2*wh_c)
                #   6-8: ph_c   (lhsT 2*wl_c)
                #   9-14: sqh_c, sql_c interleaved (lhsT -1)
                # giving D = 2 w.p - |p|^2 to ~fp32 accuracy (wl*pl dropped).
                T = sq_pool.tile([96, N * 3 // 96], f32, tag="T")  # [96, 256]
                nc.sync.dma_start(
                    out=T[:], in_=pt_flat[b].rearrange("(p f) -> p f", p=96)
                )
                Th = sq_pool.tile([96, 256], f32r, tag="Th")
                Tl = sq_pool.tile([96, 256], f32r, tag="Tl")
                Tsq = sq_pool.tile([96, 256], f32, tag="Tsq")
                Tsqh = sq_pool.tile([96, 256], f32r, tag="Tsqh")
                Tsql = sq_pool.tile([96, 256], f32r, tag="Tsql")
                nc.scalar.copy(Th[:], T[:])
                nc.gpsimd.tensor_sub(Tl[:], T[:], Th[:])
                nc.scalar.activation(Tsq[:], T[:], mybir.ActivationFunctionType.Square)
                nc.scalar.copy(Tsqh[:], Tsq[:])
                nc.gpsimd.tensor_sub(Tsql[:], Tsq[:], Tsqh[:])

                # R rows: 0-2 = ph, 3-5 = pl, 6-8 = ph again, 9-11 = sqh,
                # 12-14 = sql. Each [96,256] tile flattens to [3, 8192].
                R = rhs_pool.tile([15, N], f32r, tag="R")
                nc.sync.dma_start(out=R[0:3, :], in_=Th[:])
                nc.gpsimd.dma_start(out=R[3:6, :], in_=Tl[:])
                nc.sync.dma_start(out=R[6:9, :], in_=Th[:])
                nc.sync.dma_start(out=R[9:12, :], in_=Tsqh[:])
                nc.gpsimd.dma_start(out=R[12:15, :], in_=Tsql[:])

                # lhsT rows per group: 0-2 = 2wh, 3-5 = 2wh, 6-8 = 2wl, 9-14 = -1
                Wt = small_pool.tile([3, W], f32, tag="Wt")
                nc.sync.dma_start(out=Wt[:], in_=wt[b])
                Wh = small_pool.tile([3, W], f32r, tag="Wh")
                Wl = small_pool.tile([3, W], f32r, tag="Wl")
                nc.scalar.copy(Wh[:], Wt[:])
                nc.gpsimd.tensor_sub(Wl[:], Wt[:], Wh[:])
                Wh2 = small_pool.tile([3, W], f32r, tag="Wh2")
                Wl2 = small_pool.tile([3, W], f32r, tag="Wl2")
                nc.scalar.mul(Wh2[:], Wh[:], 2.0)
                nc.scalar.mul(Wl2[:], Wl[:], 2.0)
                Neg1f = small_pool.tile([6, W], f32, tag="Neg1f")
                Neg1 = small_pool.tile([6, W], f32r, tag="Neg1")
                nc.vector.memset(Neg1f[:], -1.0)
                nc.scalar.copy(Neg1[:], Neg1f[:])
                LT = lt_pool.tile([15, W], f32r, tag="LT")
                nc.gpsimd.dma_start(out=LT[0:3, :], in_=Wh2[:])
                nc.gpsimd.dma_start(out=LT[3:6, :], in_=Wh2[:])
                nc.gpsimd.dma_start(out=LT[6:9, :], in_=Wl2[:])
                nc.gpsimd.dma_start(out=LT[9:15, :], in_=Neg1[:])

                for wc in range(WCHUNKS):
                    Dm = d_pool.tile([128, N], f32, tag="D")
                    packs = []
                    for pk in range(4 if stage != "prep" else 0):
                        PS = psum_pool.tile([128, 2048], f32, tag="ps")
                        packs.append(PS)
                        for i in range(4):
                            nc.tensor.matmul(
                                PS[:, 512 * i:512 * (i + 1)],
                                lhsT=LT[:, wc * 128:(wc + 1) * 128],
                                rhs=R[:, (pk * 4 + i) * 512:(pk * 4 + i + 1) * 512],
                                start=True,
                                stop=True,
                            )
                        if stage not in ("mm", "p1"):
                            nc.scalar.copy(
                                Dm[:, pk * 2048:(pk + 1) * 2048], PS[:]
                            )
                    m8 = small_pool.tile([128, 8], f32, tag="m8")
                    i8 = small_pool.tile([128, 8], mybir.dt.uint32, tag="i8")
                    if stage == "prep":
                        nc.vector.max(m8[0:15, :], R[:, 0:2048].bitcast(f32))
                        nc.vector.memset(i8[:], 0)
                    elif stage == "mm":
                        nc.vector.max(m8[:], packs[3][:, 0:2048])
                        nc.vector.memset(i8[:], 0)
                    elif stage == "copy":
                        nc.vector.max(m8[:], Dm[:, 0:2048])
                        nc.vector.memset(i8[:], 0)
                    else:
                        # pass 1: per-pack max straight from PSUM (overlaps the
                        # ACT copies); combine; pass 2: one max_index over Dm.
                        for pk in range(4):
                            nc.vector.tensor_reduce(
                                m8[:, pk:pk + 1], packs[pk][:],
                                axis=mybir.AxisListType.X, op=mybir.AluOpType.max,
                            )
                        g = small_pool.tile([128, 1], f32, tag="g")
                        nc.vector.tensor_reduce(
                            g[:], m8[:, 0:4],
                            axis=mybir.AxisListType.X, op=mybir.AluOpType.max,
                        )
                        G8 = small_pool.tile([128, 8], f32, tag="G8")
                        nc.vector.tensor_copy(G8[:], g[:, 0:1].to_broadcast([128, 8]))
                        if stage in ("max8", "p1"):
                            nc.vector.memset(i8[:], 0)
                        else:
                            nc.vector.max_index(i8[:], G8[:], Dm[:])
                    nc.sync.dma_start(
                        out=idx[b, wc * 128:(wc + 1) * 128], in_=i8[:, 0:1]
                    )
    return nc


def _exp_relu(x):
    return np.where(
        x >= 0.0,
        ALPHA * np.exp(x, dtype=np.float32) - np.float32(1.0),
        np.exp(np.float32(BETA) * x, dtype=np.float32) - np.float32(1.0),
    ).astype(np.float32)


def kernel(waypointslocal, boundarypoints, boundarynormals):
    _install_workarounds()

    wp = np.ascontiguousarray(waypointslocal, dtype=np.float32)
    bp = np.ascontiguousarray(boundarypoints, dtype=np.float32)
    bn = np.ascontiguousarray(boundarynormals, dtype=np.float32)

    if "nc" not in _module_cache:
        _module_cache["nc"] = _build_module()
    nc = _module_cache["nc"]

    # shard over batch: core c gets batches [c*4, (c+1)*4)
    wt = np.ascontiguousarray(
        wp.reshape(N_CORES, B_LOC, W, 3).transpose(0, 1, 3, 2)
    )
    pt = np.ascontiguousarray(
        bp.reshape(N_CORES, B_LOC, N, 3).transpose(0, 1, 3, 2)
    )
    in_maps = [{"wt": wt[c], "pt": pt[c]} for c in range(N_CORES)]
    res = run_bass_kernel_spmd(nc, in_maps, core_ids=list(range(N_CORES)))
    idx = np.concatenate(
        [res.results[c]["idx"] for c in range(N_CORES)], axis=0
    ).astype(np.int32)  # [B, W]

    # host tail: O(B*W) gather + dot + exp_relu + mean (matches reference ops)
    cp = np.take_along_axis(bp, idx[..., None].astype(np.int64), axis=1)
    cn = np.take_along_axis(bn, idx[..., None].astype(np.int64), axis=1)
    dots = np.sum((wp - cp) * cn, axis=2, dtype=np.float32)
    relu = _exp_relu(dots)
    loss = np.mean(np.mean(relu, axis=1, dtype=np.float32), dtype=np.float32)
    return idx, np.float32(loss)


# revision 28
# speedup vs baseline: 1.2047x; 1.2047x over previous
"""BoundaryLoss kernel for Trainium2 (8 NeuronCores, SPMD data-parallel over batch).

Problem (per reference):
  B=32, W=256, N=8192, d=3
  sq[b,w,n] = |w|^2 - 2 w.p + |p|^2 ; idx = argmin_n sq ; gather closest
  point/normal; dot = (w - cp).cn ; loss = mean_b mean_w exp_relu(dot).

Device strategy per core (4 batches/core):
  D[w,n] = 2 w.p - |p|^2  (argmax_n D == argmin_n sq) via one K=15 float32r
  matmul per [128, 512] tile. f32r keeps 12 mantissa bits and streams at
  1 cycle/column; an exact hi/lo split (x = xh + xl, 12+12 = 24 bits)
  reconstructs full fp32 accuracy from three cross products (wl*pl term
  ~2^-24 is dropped):
    rhs rows 0-2 ph_c (lhsT 2*wh_c), 3-5 pl_c (lhsT 2*wh_c),
    6-8 ph_c (lhsT 2*wl_c), 9-11 sqh_c and 12-14 sql_c (lhsT -1).
  Squares/splits are computed on device in a [96, 256] view and DMA-
  flattened to [3, 8192] rhs rows. Argmin: DVE per-pack reduce_max from
  PSUM + one max_index over the ACT-drained [128, 8192] SBUF block.
  Host does the final (tiny, O(B*W)) gather + dot + exp_relu + mean.
"""

import json

import numpy as np

import concourse.bass as bass
import concourse.mybir as mybir
from concourse.tile import TileContext
from concourse.vector_clock import ScopedClock
from concourse.bass_utils import run_bass_kernel_spmd

# ---------------------------------------------------------------------------
# Workarounds: this container's walrus build rejects >1 sync-wait command per
# instruction. Split excess waits onto NoOps inserted before the instruction.
# ---------------------------------------------------------------------------

_MAX_WAITS = 1
_split_counter = [0]
_installed = [False]


def _split_bir_waits(bir_bytes):
    mod = json.loads(bir_bytes)
    changed = False
    for fn in mod.get("functions", []):
        for blk in fn.get("blocks", []):
            out = []
            for inst in blk.get("instructions", []):
                si = inst.get("sync_info")
                waits = (si or {}).get("on_wait") or []
                if len(waits) > _MAX_WAITS:
                    changed = True
                    extra, keep = waits[:-_MAX_WAITS], waits[-_MAX_WAITS:]
                    for i in range(0, len(extra), _MAX_WAITS):
                        _split_counter[0] += 1
                        out.append({
                            "engine": inst["engine"],
                            "ins": [],
                            "name": f"I-WSPLIT-{_split_counter[0]}",
                            "opcode": "NoOp",
                            "outs": [],
                            "sync_info": {
                                "on_update": [],
                                "on_wait": extra[i:i + _MAX_WAITS],
                            },
                        })
                    si["on_wait"] = keep
                out.append(inst)
            blk["instructions"] = out
    if not changed:
        return bir_bytes
    return json.dumps(mod).encode()


def _patched_drain_and_barrier(self, tick_clock, wait_clock):
    nc = self.nc
    collector = nc.sync.nop(nofuse=True, hint="drain_wait_collector")
    wait_clock.add_sem_waits(
        collector.ins, ScopedClock({None: tick_clock.global_clock})
    )
    si = collector.ins.sync_info
    if si is not None and si.on_wait and len(si.on_wait) > 1:
        waits = list(si.on_wait)
        collector.ins.sync_info = mybir.SyncInfo(
            on_wait=waits[:1], on_update=list(si.on_update)
        )
        for i in range(1, len(waits)):
            extra = nc.sync.nop(nofuse=True, hint=f"drain_wait_{i}")
            extra.ins.sync_info = mybir.SyncInfo(on_wait=[waits[i]], on_update=[])
    nc.sync.drain()
    nc.all_engine_barrier()
    assert self.sems is not None
    popped = nc._tile_sem_poison_stack.pop()
    assert popped is self._sem_poison
    nc.clear_and_free_semaphores(list(self.sems.allocated().values()))
    nc.all_engine_barrier()


def _install_workarounds():
    if _installed[0]:
        return
    _installed[0] = True
    TileContext._drain_and_barrier = _patched_drain_and_barrier

    import concourse.bass2jax as bass2jax

    orig_compile = bass2jax.compile_bir_kernel

    def compile_with_split(ant_bir_str, compile_dir_path, neff_name="file.neff", **kw):
        if isinstance(ant_bir_str, str):
            fixed = _split_bir_waits(ant_bir_str.encode()).decode()
        else:
            fixed = _split_bir_waits(ant_bir_str)
        return orig_compile(fixed, compile_dir_path, neff_name=neff_name, **kw)

    bass2jax.compile_bir_kernel = compile_with_split


# ---------------------------------------------------------------------------
# Problem constants (hardcoded per the harness contract)
# ---------------------------------------------------------------------------

B, W, N, D3 = 32, 256, 8192, 3
N_CORES = 8
B_LOC = B // N_CORES          # 4 batches per core
WCHUNKS = W // 128            # 2 row blocks of 128 waypoints
NCHUNK = 512                  # moving-operand / PSUM bank width
NCHUNKS = N // NCHUNK         # 16

ALPHA = 1.0
BETA = 0.5

_module_cache = {}


def _build_module(loop_reps=None, stage="full"):
    """One NeuronCore's module (SPMD; 4 batches per core).

    f32r (12-bit mantissa) hi/lo-split K=15 matmuls compute the negated
    squared distance D = 2 w.p - |p|^2 at full fp32 accuracy but 1 cycle/
    column (4x faster than fp32 matmul). DVE does a per-pack max from PSUM
    plus one max_index over the assembled SBUF row block; ACT drains
    PSUM->SBUF; prep DMAs are spread over SP/Pool DGE queues.

    loop_reps unrolls the body for amplified timing; stage carves out
    pipeline subsets for bisection benchmarks. Production: defaults."""
    f32 = mybir.dt.float32
    nc = bass.Bass()
    wt = nc.dram_tensor("wt", [B_LOC, 3, W], f32, kind="ExternalInput")
    pt = nc.dram_tensor("pt", [B_LOC, 3, N], f32, kind="ExternalInput")
    idx = nc.dram_tensor("idx", [B_LOC, W], mybir.dt.uint32, kind="ExternalOutput")

    pt_flat = pt.rearrange("b c n -> b (c n)")

    f32r = mybir.dt.float32r
    K15 = 15  # contraction rows per group (f32r hi/lo decomposition)

    with TileContext(nc) as tc:
        with (
            tc.tile_pool(name="rhs", bufs=2) as rhs_pool,
            tc.tile_pool(name="sq", bufs=2) as sq_pool,
            tc.tile_pool(name="lt", bufs=2) as lt_pool,
            tc.tile_pool(name="dmat", bufs=3) as d_pool,
            tc.tile_pool(name="small", bufs=4) as small_pool,
            tc.tile_pool(name="psum", bufs=2, space="PSUM") as psum_pool,
        ):
          for _rep in range(loop_reps or 1):
            for b in range(B_LOC):
                # f32r hi/lo split: x = xh + xl exactly (12+12 mantissa bits).
                # Per row group (partition offset 32i) the K=15 rhs rows are:
                #   0-2: ph_c   (lhsT 2*wh_c)
                #   3-5: pl_c   (lhsT 2*wh_c)
                #   6-8: ph_c   (lhsT 2*wl_c)
                #   9-14: sqh_c, sql_c interleaved (lhsT -1)
                # giving D = 2 w.p - |p|^2 to ~fp32 accuracy (wl*pl dropped).
                T = sq_pool.tile([96, N * 3 // 96], f32, tag="T")  # [96, 256]
                nc.sync.dma_start(
                    out=T[:], in_=pt_flat[b].rearrange("(p f) -> p f", p=96)
                )
                Th = sq_pool.tile([96, 256], f32r, tag="Th")
                Tl = sq_pool.tile([96, 256], f32r, tag="Tl")
                Tsq = sq_pool.tile([96, 256], f32, tag="Tsq")
                Tsqh = sq_pool.tile([96, 256], f32r, tag="Tsqh")
                Tsql = sq_pool.tile([96, 256], f32r, tag="Tsql")
                nc.scalar.copy(Th[:], T[:])
                nc.gpsimd.tensor_sub(Tl[:], T[:], Th[:])
                nc.scalar.activation(Tsq[:], T[:], mybir.ActivationFunctionType.Square)
                nc.scalar.copy(Tsqh[:], Tsq[:])
                nc.gpsimd.tensor_sub(Tsql[:], Tsq[:], Tsqh[:])

                # R rows: 0-2 = ph, 3-5 = pl, 6-8 = ph again, 9-11 = sqh,
                # 12-14 = sql. Each [96,256] tile flattens to [3, 8192].
                R = rhs_pool.tile([15, N], f32r, tag="R")
                nc.sync.dma_start(out=R[0:3, :], in_=Th[:])
                nc.gpsimd.dma_start(out=R[3:6, :], in_=Tl[:])
                nc.sync.dma_start(out=R[6:9, :], in_=Th[:])
                nc.sync.dma_start(out=R[9:12, :], in_=Tsqh[:])
                nc.gpsimd.dma_start(out=R[12:15, :], in_=Tsql[:])

                # lhsT rows per group: 0-2 = 2wh, 3-5 = 2wh, 6-8 = 2wl, 9-14 = -1
                Wt = small_pool.tile([3, W], f32, tag="Wt")
                nc.sync.dma_start(out=Wt[:], in_=wt[b])
                Wh = small_pool.tile([3, W], f32r, tag="Wh")
                Wl = small_pool.tile([3, W], f32r, tag="Wl")
                nc.scalar.copy(Wh[:], Wt[:])
                nc.gpsimd.tensor_sub(Wl[:], Wt[:], Wh[:])
                Wh2 = small_pool.tile([3, W], f32r, tag="Wh2")
                Wl2 = small_pool.tile([3, W], f32r, tag="Wl2")
                nc.scalar.mul(Wh2[:], Wh[:], 2.0)
                nc.scalar.mul(Wl2[:], Wl[:], 2.0)
                Neg1f = small_pool.tile([6, W], f32, tag="Neg1f")
                Neg1 = small_pool.tile([6, W], f32r, tag="Neg1")
                nc.vector.memset(Neg1f[:], -1.0)
                nc.scalar.copy(Neg1[:], Neg1f[:])
                LT = lt_pool.tile([15, W], f32r, tag="LT")
                nc.gpsimd.dma_start(out=LT[0:3, :], in_=Wh2[:])
                nc.gpsimd.dma_start(out=LT[3:6, :], in_=Wh2[:])
                nc.gpsimd.dma_start(out=LT[6:9, :], in_=Wl2[:])
                nc.gpsimd.dma_start(out=LT[9:15, :], in_=Neg1[:])

                for wc in range(WCHUNKS):
                    Dm = d_pool.tile([128, N], f32, tag="D")
                    packs = []
                    for pk in range(4 if stage != "prep" else 0):
                        PS = psum_pool.tile([128, 2048], f32, tag="ps")
                        packs.append(PS)
                        for i in range(4):
                            nc.tensor.matmul(
                                PS[:, 512 * i:512 * (i + 1)],
                                lhsT=LT[:, wc * 128:(wc + 1) * 128],
                                rhs=R[:, (pk * 4 + i) * 512:(pk * 4 + i + 1) * 512],
                                start=True,
                                stop=True,
                            )
                        if stage not in ("mm", "p1"):
                            nc.scalar.copy(
                                Dm[:, pk * 2048:(pk + 1) * 2048], PS[:]
                            )
                    m8 = small_pool.tile([128, 8], f32, tag="m8")
                    i8 = small_pool.tile([128, 8], mybir.dt.uint32, tag="i8")
                    if stage == "prep":
                        nc.vector.max(m8[0:15, :], R[:, 0:2048].bitcast(f32))
                        nc.vector.memset(i8[:], 0)
                    elif stage == "mm":
                        nc.vector.max(m8[:], packs[3][:, 0:2048])
                        nc.vector.memset(i8[:], 0)
                    elif stage == "copy":
                        nc.vector.max(m8[:], Dm[:, 0:2048])
                        nc.vector.memset(i8[:], 0)
                    else:
                        # top-8 then index of the max over the SBUF row block
                        nc.vector.max(m8[:], Dm[:])
                        if stage in ("max8", "p1"):
                            nc.vector.memset(i8[:], 0)
                        else:
                            nc.vector.max_index(i8[:], m8[:], Dm[:])
                    nc.sync.dma_start(
                        out=idx[b, wc * 128:(wc + 1) * 128], in_=i8[:, 0:1]
                    )
    return nc


def _exp_relu(x):
    return np.where(
        x >= 0.0,
        ALPHA * np.exp(x, dtype=np.float32) - np.float32(1.0),
        np.exp(np.float32(BETA) * x, dtype=np.float32) - np.float32(1.0),
    ).astype(np.float32)


def kernel(waypointslocal, boundarypoints, boundarynormals):
    _install_workarounds()

    wp = np.ascontiguousarray(waypointslocal, dtype=np.float32)
    bp = np.ascontiguousarray(boundarypoints, dtype=np.float32)
    bn = np.ascontiguousarray(boundarynormals, dtype=np.float32)

    if "nc" not in _module_cache:
        _module_cache["nc"] = _build_module()
    nc = _module_cache["nc"]

    # shard over batch: core c gets batches [c*4, (c+1)*4)
    wt = np.ascontiguousarray(
        wp.reshape(N_CORES, B_LOC, W, 3).transpose(0, 1, 3, 2)
    )
    pt = np.ascontiguousarray(
        bp.reshape(N_CORES, B_LOC, N, 3).transpose(0, 1, 3, 2)
    )
    in_maps = [{"wt": wt[c], "pt": pt[c]} for c in range(N_CORES)]
    res = run_bass_kernel_spmd(nc, in_maps, core_ids=list(range(N_CORES)))
    idx = np.concatenate(
        [res.results[c]["idx"] for c in range(N_CORES)], axis=0
    ).astype(np.int32)  # [B, W]

    # host tail: O(B*W) gather + dot + exp_relu + mean (matches reference ops)
    cp = np.take_along_axis(bp, idx[..., None].astype(np.int64), axis=1)
    cn = np.take_along_axis(bn, idx[..., None].astype(np.int64), axis=1)
    dots = np.sum((wp - cp) * cn, axis=2, dtype=np.float32)
    relu = _exp_relu(dots)
    loss = np.mean(np.mean(relu, axis=1, dtype=np.float32), dtype=np.float32)
    return idx, np.float32(loss)
